# revision 1
# baseline (speedup 1.0000x reference)
"""Trainium2 Bass kernel for nn_Decoder (LSTM decoder + vocab projection).

Reference computation (B=64, S=64, E=256, H=512, V=32000):
    emb     = emb_table[target_seq]                      [B,S,E]
    lstm_in = concat([emb, ctx_broadcast], -1)           [B,S,E+H]
    pre     = lstm_in @ w_ih.T + b_ih + b_hh             [B,S,4H]
    per step: gates = pre_t + h @ w_hh.T ; LSTM update   [B,4H]
    logits  = concat([hs, ctx], -1) @ w_fc.T + b_fc      [B,S,V]

Sharding (8 cores):
  - batch 8-way for the embedding/pre/LSTM recurrence (8 batches/core)
  - vocab 8-way for the FC projection (w_fc shard resident in SBUF)
  - hidden states AllGathered across cores in 4 blocks of 16 steps
  - FC uses the split  logits = hs @ w_fc[:, :H].T + (ctx @ w_fc[:, H:].T
    + b_fc)  where the ctx part is computed ONCE per batch (not per step),
    halving FC FLOPs.

Everything on device is computed "transposed": states / gates keep the
feature dim on SBUF partitions, batch on the free dim.  Gate blocks are
host-permuted to [i, f, o, g] order so sigmoid/tanh each cover one
contiguous slice.

Token indexing: local token n = t*8 + b_local (t-major).  Global gathered
token tau = block*1024 + core*128 + (t%16)*8 + b_local.
"""

import numpy as np
import ml_dtypes

VOCAB, EMBED, HIDDEN = 32000, 256, 512
B, S = 64, 64
NCORES = 8
BL = B // NCORES          # 8 local batches
TOKL = S * BL             # 512 local tokens
TOK = S * B               # 4096 global tokens
G4 = 4 * HIDDEN           # 2048
KIN = EMBED + HIDDEN      # 768
KI = KIN // 128           # 6 k-tiles for pre
KH = HIDDEN // 128        # 4 k-tiles for recurrence / FC
GT = G4 // 128            # 16 gate tiles
VSH = 4096                # per-core (overlapping) vocab shard
VT = VSH // 128           # 32 vocab tiles
BLOCKS = [16, 16, 16, 8, 8]   # allgather block sizes (steps); small tail
NBLK = len(BLOCKS)
BSTART = [sum(BLOCKS[:j]) for j in range(NBLK)]      # first step of block
TAUB = [sum(BLOCKS[:j]) * B for j in range(NBLK)]    # tau base of block

V_STARTS = [0, 4000, 8000, 12000, 16000, 20000, 24000, VOCAB - VSH]

BF16 = ml_dtypes.bfloat16
FP8 = ml_dtypes.float8_e4m3

_CACHE = {}


def _build_program():
    import concourse.bass as bass
    import concourse.mybir as mybir
    import concourse.tile as tile
    from concourse import bacc

    bf = mybir.dt.bfloat16
    f8 = mybir.dt.float8e4
    f32 = mybir.dt.float32
    AF = mybir.ActivationFunctionType

    nc = bacc.Bacc(
        "TRN2",
        target_bir_lowering=False,
        debug=False,
        num_devices=NCORES,
    )

    # ---- DRAM I/O ----------------------------------------------------
    x_d = nc.dram_tensor("x_d", [KI, 128, TOKL], bf, kind="ExternalInput").ap()
    wih_d = nc.dram_tensor("wih_d", [KI, 128, G4], bf, kind="ExternalInput").ap()
    whh_d = nc.dram_tensor("whh_d", [KH, 128, G4], f8, kind="ExternalInput").ap()
    bias_d = nc.dram_tensor("bias_d", [128, GT], f32, kind="ExternalInput").ap()
    h0_d = nc.dram_tensor("h0_d", [128, KH, BL], bf, kind="ExternalInput").ap()
    c0_d = nc.dram_tensor("c0_d", [128, KH, BL], f32, kind="ExternalInput").ap()
    ctx_d = nc.dram_tensor("ctx_d", [KH, 128, B], bf, kind="ExternalInput").ap()
    wfh_d = nc.dram_tensor("wfh_d", [KH, 128, VSH], bf, kind="ExternalInput").ap()
    wfc_d = nc.dram_tensor("wfc_d", [KH, 128, VSH], bf, kind="ExternalInput").ap()
    bfc_d = nc.dram_tensor("bfc_d", [128, VT], f32, kind="ExternalInput").ap()
    log_d = nc.dram_tensor("log_d", [VSH, TOK], f32, kind="ExternalOutput").ap()

    with tile.TileContext(nc) as tc, \
         tc.tile_pool(name="singles", bufs=1) as sg, \
         tc.tile_pool(name="dramb", bufs=1, space="DRAM") as dramb:
        # ---- persistent SBUF tensors ---------------------------------
        x_sb = sg.tile([128, KI, TOKL], bf, name="x_sb", tag="x_sb")
        wih_sb = sg.tile([128, KI, G4], bf, name="wih_sb", tag="wih_sb")
        whh_sb = sg.tile([128, KH, G4], f8, name="whh_sb", tag="whh_sb")
        bias_sb = sg.tile([128, GT], f32, name="bias_sb", tag="bias_sb")
        h0_sb = sg.tile([128, KH, BL], bf, name="h0_sb", tag="h0_sb")
        c0_sb = sg.tile([128, KH, BL], f32, name="c0_sb", tag="c0_sb")
        ctx_sb = sg.tile([128, KH, B], bf, name="ctx_sb", tag="ctx_sb")
        wfh_sb = sg.tile([128, KH, VSH], bf, name="wfh_sb", tag="wfh_sb")
        wfc_sb = sg.tile([128, KH, VSH], bf, name="wfc_sb", tag="wfc_sb")
        bfc_sb = sg.tile([128, VT], f32, name="bfc_sb", tag="bfc_sb")
        pre_sb = sg.tile([128, GT, TOKL], bf, name="pre_sb", tag="pre_sb")
        ctxl_sb = sg.tile([128, VT, B], f32, name="ctxl_sb", tag="ctxl_sb")
        hs_sb = sg.tile([128, KH, TOK], bf, name="hs_sb", tag="hs_sb")
        stages = [
            sg.tile([128, KH, BLOCKS[j] * BL], bf, name=f"stage{j}",
                    tag=f"stage{j}")
            for j in range(NBLK)
        ]

        # ---- input DMAs ---------------------------------------------
        # spread input loads across queues so SP isn't a startup bottleneck
        nc.sync.dma_start(out=x_sb[:], in_=x_d.rearrange("k p n -> p k n"))
        nc.sync.dma_start(out=wih_sb[:], in_=wih_d.rearrange("k p n -> p k n"))
        nc.sync.dma_start(out=whh_sb[:], in_=whh_d.rearrange("k p n -> p k n"))
        nc.gpsimd.dma_start(out=bias_sb[:], in_=bias_d)
        nc.gpsimd.dma_start(out=h0_sb[:], in_=h0_d)
        nc.gpsimd.dma_start(out=c0_sb[:], in_=c0_d)
        nc.gpsimd.dma_start(out=ctx_sb[:], in_=ctx_d.rearrange("k p n -> p k n"))
        nc.gpsimd.dma_start(out=wfc_sb[:], in_=wfc_d.rearrange("k p n -> p k n"))
        nc.gpsimd.dma_start(out=wfh_sb[:], in_=wfh_d.rearrange("k p n -> p k n"))
        nc.gpsimd.dma_start(out=bfc_sb[:], in_=bfc_d)

        # ---- DRAM bounce buffers for the AllGathers ------------------
        ccin = []
        ccout = []
        for j in range(NBLK):
            bn = BLOCKS[j] * BL
            ci = dramb.tile([KH, 128, bn], bf, name=f"ccin{j}",
                            tag=f"ccin{j}")
            co = dramb.tile(
                [NCORES, KH, 128, bn], bf,
                addr_space="Shared", name=f"ccout{j}", tag=f"ccout{j}",
            )
            ccin.append(ci)
            ccout.append(co)

        with (
            tc.tile_pool(name="pmm", bufs=4, space="PSUM") as pmm,
            tc.tile_pool(name="pgate", bufs=2, space="PSUM") as pgate,
            tc.tile_pool(name="act", bufs=3) as actp,
            tc.tile_pool(name="cst", bufs=3) as cstp,
            tc.tile_pool(name="fout", bufs=4) as foutp,
        ):
            # ---- phase 1: pre = x @ w_ih.T + bias (transposed) -------
            for gt in range(GT):
                ps = pmm.tile([128, TOKL], f32, tag="mm512")
                for kt in range(KI):
                    nc.tensor.matmul(
                        ps[:],
                        lhsT=wih_sb[:, kt, gt * 128:(gt + 1) * 128],
                        rhs=x_sb[:, kt, :],
                        start=(kt == 0),
                        stop=(kt == KI - 1),
                    )
                nc.scalar.activation(
                    pre_sb[:, gt], ps[:], AF.Identity,
                    bias=bias_sb[:, gt:gt + 1],
                )

            # ---- phase 3: LSTM recurrence ----------------------------
            def blk_of(t):
                for j in range(NBLK):
                    if t < BSTART[j] + BLOCKS[j]:
                        return j, t - BSTART[j]
                raise AssertionError(t)

            c_prev = c0_sb
            for t in range(S):
                j, t16 = blk_of(t)
                if t == 0:
                    rhs_src = h0_sb
                    roff = 0
                else:
                    pj, pt = blk_of(t - 1)
                    rhs_src = stages[pj]
                    roff = pt * BL

                gp = pgate.tile([128, GT, BL], f32, tag="gates")
                for gt in range(GT):
                    for kt in range(KH):
                        nc.tensor.matmul(
                            gp[:, gt],
                            lhsT=whh_sb[:, kt, gt * 128:(gt + 1) * 128],
                            rhs=rhs_src[:, kt, roff:roff + BL],
                            start=(kt == 0),
                            stop=(kt == KH - 1),
                        )
                # add precomputed input contribution (in-place in PSUM)
                nc.vector.tensor_add(
                    gp[:], gp[:],
                    pre_sb[:, :, t * BL:(t + 1) * BL],
                )
                # activations: blocks are [i,f,o | g] after host permute
                sig = actp.tile([128, 3 * KH, BL], bf, tag="sig")
                gg = actp.tile([128, KH, BL], bf, tag="gg")
                nc.scalar.activation(sig[:], gp[:, 0:3 * KH], AF.Sigmoid)
                nc.scalar.activation(gg[:], gp[:, 3 * KH:GT], AF.Tanh)

                ig = cstp.tile([128, KH, BL], f32, tag="ig")
                fc = cstp.tile([128, KH, BL], f32, tag="fc")
                c_new = cstp.tile([128, KH, BL], f32, tag="c")
                tcn = cstp.tile([128, KH, BL], bf, tag="tc")
                nc.vector.tensor_mul(ig[:], sig[:, 0:KH], gg[:])
                nc.vector.tensor_mul(fc[:], sig[:, KH:2 * KH], c_prev[:])
                nc.vector.tensor_add(c_new[:], ig[:], fc[:])
                nc.scalar.activation(tcn[:], c_new[:], AF.Tanh)
                # write h in two halves so step t+1's k0/k1 matmuls can
                # start before the second half lands
                half = KH // 2
                st = stages[j][:, :, t16 * BL:(t16 + 1) * BL]
                nc.vector.tensor_mul(
                    st[:, 0:half], sig[:, 2 * KH:2 * KH + half],
                    tcn[:, 0:half],
                )
                nc.vector.tensor_mul(
                    st[:, half:KH], sig[:, 2 * KH + half:3 * KH],
                    tcn[:, half:KH],
                )
                c_prev = c_new

                if t16 == BLOCKS[j] - 1:
                    # ship this block: stage -> DRAM -> AllGather -> SBUF
                    bn = BLOCKS[j] * BL
                    nc.sync.dma_start(
                        out=ccin[j][:].rearrange("k p n -> p k n"),
                        in_=stages[j][:],
                    )
                    nc.gpsimd.collective_compute(
                        "AllGather",
                        mybir.AluOpType.bypass,
                        replica_groups=[list(range(NCORES))],
                        ins=[ccin[j][:]],
                        outs=[ccout[j][:]],
                    )
                    for c in range(NCORES):
                        nc.sync.dma_start(
                            out=hs_sb[:, :, TAUB[j] + c * bn:
                                      TAUB[j] + (c + 1) * bn],
                            in_=ccout[j][c].rearrange("k p n -> p k n"),
                        )

            # ---- phase 2 (emitted late = low priority; only needed by
            # the FC): ctx logits  ctx @ w_fc[:,H:].T + b_fc ----------
            for vt in range(VT):
                ps = pmm.tile([128, B], f32, tag="mm512")
                for kt in range(KH):
                    nc.tensor.matmul(
                        ps[:],
                        lhsT=wfc_sb[:, kt, vt * 128:(vt + 1) * 128],
                        rhs=ctx_sb[:, kt, :],
                        start=(kt == 0),
                        stop=(kt == KH - 1),
                    )
                nc.scalar.activation(
                    ctxl_sb[:, vt], ps[:], AF.Identity,
                    bias=bfc_sb[:, vt:vt + 1],
                )

            # ---- phase 4: FC  hs @ w_fc[:,:H].T + ctxl ---------------
            dma_engines = [nc.sync, nc.scalar, nc.gpsimd]
            ndma = 0
            for j in range(NBLK):
                W = BLOCKS[j] * B          # tau width of this block
                nch = W // 512             # 512-token chunks
                cpc = NCORES // nch        # cores covered per chunk
                for vt in range(VT):
                    fo = foutp.tile([128, 1024], f32, tag="fo")
                    for m in range(nch):
                        n0 = TAUB[j] + m * 512
                        ps = pmm.tile([128, 512], f32, tag="mm512")
                        for kt in range(KH):
                            nc.tensor.matmul(
                                ps[:],
                                lhsT=wfh_sb[:, kt, vt * 128:(vt + 1) * 128],
                                rhs=hs_sb[:, kt, n0:n0 + 512],
                                start=(kt == 0),
                                stop=(kt == KH - 1),
                            )
                        ctxv = (
                            ctxl_sb[:, vt, m * cpc * BL:(m + 1) * cpc * BL]
                            .rearrange("p (c b) -> p c b", b=BL)
                            .unsqueeze(2)
                            .broadcast_to([128, cpc, BLOCKS[j], BL])
                        )
                        nc.vector.tensor_add(
                            fo[:, m * 512:(m + 1) * 512]
                            .rearrange("p (c t b) -> p c t b",
                                       c=cpc, t=BLOCKS[j], b=BL),
                            ps[:].rearrange("p (c t b) -> p c t b",
                                            c=cpc, t=BLOCKS[j], b=BL),
                            ctxv,
                        )
                    # one store per (block, vtile); spread the dispatch
                    # cost across SP / ACT / Pool queues
                    eng = dma_engines[ndma % 3]
                    ndma += 1
                    eng.dma_start(
                        out=log_d[vt * 128:(vt + 1) * 128, TAUB[j]:TAUB[j] + W],
                        in_=fo[:, 0:W],
                    )

    nc.compile()
    return nc


def _get_nc():
    if "nc" not in _CACHE:
        _CACHE["nc"] = _build_program()
    return _CACHE["nc"]


def _block128(a):
    """[K, N] -> [K//128, 128, N] contiguous blocks."""
    k, n = a.shape
    return np.ascontiguousarray(a.reshape(k // 128, 128, n))


def _t_layout(a):
    """[BL, 512] state -> [128, KH, BL] transposed tile layout."""
    # out[p, kt, b] = a[b, kt*128 + p]
    return np.ascontiguousarray(a.T.reshape(KH, 128, BL).transpose(1, 0, 2))


def _prep_in_maps(target_seq, context, h, c, emb_table, w_ih, w_hh, b_ih,
                  b_hh, w_fc, b_fc):
    target_seq = np.asarray(target_seq)
    context = np.asarray(context, dtype=np.float32)
    h = np.asarray(h, dtype=np.float32)
    c = np.asarray(c, dtype=np.float32)
    emb_table = np.asarray(emb_table, dtype=np.float32)
    w_ih = np.asarray(w_ih, dtype=np.float32)
    w_hh = np.asarray(w_hh, dtype=np.float32)
    b_ih = np.asarray(b_ih, dtype=np.float32)
    b_hh = np.asarray(b_hh, dtype=np.float32)
    w_fc = np.asarray(w_fc, dtype=np.float32)
    b_fc = np.asarray(b_fc, dtype=np.float32)

    # gate-block permutation [i, f, g, o] -> [i, f, o, g]
    perm = np.concatenate([
        np.arange(0, HIDDEN),                    # i
        np.arange(HIDDEN, 2 * HIDDEN),           # f
        np.arange(3 * HIDDEN, 4 * HIDDEN),       # o
        np.arange(2 * HIDDEN, 3 * HIDDEN),       # g
    ])
    w_ih_p = w_ih[perm]
    w_hh_p = w_hh[perm]
    bias_p = (b_ih + b_hh)[perm]

    wih_d = _block128(w_ih_p.T.astype(BF16))          # [6,128,2048]
    whh_d = _block128(w_hh_p.T.astype(FP8))           # [4,128,2048]
    bias_d = np.ascontiguousarray(
        bias_p.reshape(GT, 128).T.astype(np.float32))  # [128,16]
    ctx_d = _block128(context.T.astype(BF16))          # [4,128,64]

    emb = emb_table[target_seq]                        # [B,S,E] f32

    in_maps = []
    for cid in range(NCORES):
        bs = slice(cid * BL, (cid + 1) * BL)
        # lstm_in transposed, local tokens n = t*8+b
        x_loc = np.concatenate(
            [
                emb[bs].transpose(1, 0, 2).reshape(TOKL, EMBED),
                np.tile(context[bs], (S, 1)),
            ],
            axis=1,
        )                                              # [512, 768]
        x_d = _block128(x_loc.T.astype(BF16))          # [6,128,512]
        vs = V_STARTS[cid]
        wfh_d = _block128(
            np.ascontiguousarray(w_fc[vs:vs + VSH, :HIDDEN].T).astype(BF16))
        wfc_d = _block128(
            np.ascontiguousarray(w_fc[vs:vs + VSH, HIDDEN:].T).astype(BF16))
        bfc_d = np.ascontiguousarray(
            b_fc[vs:vs + VSH].reshape(VT, 128).T.astype(np.float32))
        in_maps.append({
            "x_d": x_d,
            "wih_d": wih_d,
            "whh_d": whh_d,
            "bias_d": bias_d,
            "h0_d": _t_layout(h[bs]).astype(BF16),
            "c0_d": _t_layout(c[bs]).astype(np.float32),
            "ctx_d": ctx_d,
            "wfh_d": wfh_d,
            "wfc_d": wfc_d,
            "bfc_d": bfc_d,
        })
    return in_maps


def _assemble(results):
    """results: list of per-core {"log_d": [4096, 4096]} -> [B, S, V]."""
    full = np.empty((VOCAB, TOK), dtype=np.float32)
    for cid in range(NCORES):
        out_c = results[cid]["log_d"]                  # [4096, 4096]
        vs = V_STARTS[cid]
        r0 = cid * 4000 - vs
        full[cid * 4000:(cid + 1) * 4000] = out_c[r0:r0 + 4000]

    # tau = TAUB[j] + core*(BLOCKS[j]*8) + t_in*8 + b
    logits = np.empty((B, S, VOCAB), dtype=np.float32)
    for j in range(NBLK):
        W = BLOCKS[j] * B
        blk = full[:, TAUB[j]:TAUB[j] + W]             # [V, W]
        blk = blk.reshape(VOCAB, NCORES, BLOCKS[j], BL)
        # -> [core, b, t_in, V]
        logits[:, BSTART[j]:BSTART[j] + BLOCKS[j], :] = (
            blk.transpose(1, 3, 2, 0).reshape(B, BLOCKS[j], VOCAB)
        )
    return np.ascontiguousarray(logits)


def kernel(target_seq, context, h, c, emb_table, w_ih, w_hh, b_ih, b_hh,
           w_fc, b_fc):
    from concourse.bass_utils import run_bass_kernel_spmd

    in_maps = _prep_in_maps(target_seq, context, h, c, emb_table, w_ih,
                            w_hh, b_ih, b_hh, w_fc, b_fc)
    nc = _get_nc()
    res = run_bass_kernel_spmd(nc, in_maps, core_ids=list(range(NCORES)))
    return _assemble(res.results)



# revision 28
# speedup vs baseline: 1.7598x; 1.7598x over previous
"""Trainium2 Bass kernel for nn_Decoder (LSTM decoder + vocab projection).

Reference computation (B=64, S=64, E=256, H=512, V=32000):
    emb     = emb_table[target_seq]                      [B,S,E]
    lstm_in = concat([emb, ctx_broadcast], -1)           [B,S,E+H]
    pre     = lstm_in @ w_ih.T + b_ih + b_hh             [B,S,4H]
    per step: gates = pre_t + h @ w_hh.T ; LSTM update   [B,4H]
    logits  = concat([hs, ctx], -1) @ w_fc.T + b_fc      [B,S,V]

Sharding (8 cores): pure batch-parallel, 8 batches/core, NO collectives.
Each core computes the FULL vocab projection for its own 512 local
tokens (n = t*8 + b, t-major):
  - w_fc[:, :H] streamed+resident in fp8; FC matmuls use fp8 DoubleRow
    (two 128-k-tiles per instruction, 0.5 PE cycles/row).
  - ctx half: ctxl[v, b] = ctx_b @ w_fc[v, H:] + b_fc computed on
    device from a streamed bf16 w2; added to the FC PSUM during
    PSUM->SBUF evacuation as a broadcast tensor_add.
  - recurrence: h kept in fp8 (feeds both DoubleRow h@w_hh and FC rhs),
    c in f32.  Gate blocks host-permuted to [i, f, o, g].
  - logits stored bf16 [250, 128, 512] per core; host upcasts to f32.

Weight/FC instructions are interleaved into the recurrence's PE queue
(which is latency- not throughput-bound) so the vocab projection of the
first 32 steps overlaps the last 32 steps of the recurrence.
"""

import numpy as np
import ml_dtypes

VOCAB, EMBED, HIDDEN = 32000, 256, 512
B, S = 64, 64
NCORES = 8
BL = B // NCORES          # 8 local batches
TOKL = S * BL             # 512 local tokens
G4 = 4 * HIDDEN           # 2048
KIN = EMBED + HIDDEN      # 768
KI = KIN // 128           # 6 k-tiles for pre
KH = HIDDEN // 128        # 4 k-tiles
GT = G4 // 128            # 16 gate tiles
VT = VOCAB // 128         # 250 vocab tiles
VG = 5                    # vtiles per weight-stream group
NG = VT // VG             # 50 groups
HALF = TOKL // 2          # 256 tokens per FC half

BF16 = ml_dtypes.bfloat16
FP8 = ml_dtypes.float8_e4m3

_CACHE = {}


def _build_program():
    import concourse.bass as bass
    import concourse.mybir as mybir
    import concourse.tile as tile
    from concourse import bacc

    bf = mybir.dt.bfloat16
    f8 = mybir.dt.float8e4
    f32 = mybir.dt.float32
    AF = mybir.ActivationFunctionType
    DR = mybir.MatmulPerfMode.DoubleRow

    nc = bacc.Bacc(
        "TRN2",
        target_bir_lowering=False,
        debug=False,
        num_devices=NCORES,
    )

    # ---- DRAM I/O ----------------------------------------------------
    x_d = nc.dram_tensor("x_d", [KI, 128, TOKL], bf, kind="ExternalInput").ap()
    wih_d = nc.dram_tensor("wih_d", [KI, 128, G4], f8, kind="ExternalInput").ap()
    whh_d = nc.dram_tensor("whh_d", [KH, 128, G4], f8, kind="ExternalInput").ap()
    bias_d = nc.dram_tensor("bias_d", [128, GT], f32, kind="ExternalInput").ap()
    h0_d = nc.dram_tensor("h0_d", [128, KH, BL], f8, kind="ExternalInput").ap()
    c0_d = nc.dram_tensor("c0_d", [128, KH, BL], f32, kind="ExternalInput").ap()
    ctxT_d = nc.dram_tensor("ctxT_d", [KH, 128, BL], bf, kind="ExternalInput").ap()
    id_d = nc.dram_tensor("id_d", [128, 128], f8, kind="ExternalInput").ap()
    w1_d = nc.dram_tensor("w1_d", [KH, 128, VOCAB], f8, kind="ExternalInput").ap()
    w2_d = nc.dram_tensor("w2_d", [KH, 128, VOCAB], bf, kind="ExternalInput").ap()
    bfc_d = nc.dram_tensor("bfc_d", [128, VT], f32, kind="ExternalInput").ap()
    log_d = nc.dram_tensor("log_d", [VT, 128, TOKL], bf, kind="ExternalOutput").ap()

    with tile.TileContext(nc) as tc, \
         tc.tile_pool(name="singles", bufs=1) as sg:
        # ---- persistent SBUF tensors ---------------------------------
        x_sb = sg.tile([128, KI, TOKL], bf, name="x_sb", tag="x_sb")
        wih_sb = sg.tile([128, KI, G4], f8, name="wih_sb", tag="wih_sb")
        whh_sb = sg.tile([128, KH, G4], f8, name="whh_sb", tag="whh_sb")
        bias_sb = sg.tile([128, GT], f32, name="bias_sb", tag="bias_sb")
        h0_sb = sg.tile([128, KH, BL], f8, name="h0_sb", tag="h0_sb")
        c0_sb = sg.tile([128, KH, BL], f32, name="c0_sb", tag="c0_sb")
        ctxT_sb = sg.tile([128, KH, BL], bf, name="ctxT_sb", tag="ctxT_sb")
        id_sb = sg.tile([128, 128], f8, name="id_sb", tag="id_sb")
        bfc_sb = sg.tile([128, VT], f32, name="bfc_sb", tag="bfc_sb")
        pre_sb = sg.tile([128, GT, TOKL], bf, name="pre_sb", tag="pre_sb")
        hs_sb = sg.tile([128, KH, TOKL], f8, name="hs_sb", tag="hs_sb")
        ctxl_sb = sg.tile([128, VT, BL], bf, name="ctxl_sb", tag="ctxl_sb")
        w1_sb = sg.tile([128, KH, VOCAB], f8, name="w1_sb", tag="w1_sb")

        # ---- input DMAs (SP: recurrence-critical smalls first) -------
        nc.gpsimd.dma_start(out=x_sb[:], in_=x_d.rearrange("k p n -> p k n"))
        nc.gpsimd.dma_start(out=wih_sb[:], in_=wih_d.rearrange("k p n -> p k n"))
        nc.gpsimd.dma_start(out=whh_sb[:], in_=whh_d.rearrange("k p n -> p k n"))
        nc.gpsimd.dma_start(out=ctxT_sb[:], in_=ctxT_d.rearrange("k p n -> p k n"))
        nc.gpsimd.dma_start(out=bfc_sb[:], in_=bfc_d)
        nc.sync.dma_start(out=bias_sb[:], in_=bias_d)
        nc.sync.dma_start(out=h0_sb[:], in_=h0_d)
        nc.sync.dma_start(out=c0_sb[:], in_=c0_d)
        nc.sync.dma_start(out=id_sb[:], in_=id_d)

        with (
            tc.tile_pool(name="w2p", bufs=2) as w2p,
            tc.tile_pool(name="act", bufs=3) as actp,
            tc.tile_pool(name="cst", bufs=2) as cstp,
            tc.tile_pool(name="tmp", bufs=2) as tmpp,
            tc.tile_pool(name="fout", bufs=3) as foutp,
            tc.tile_pool(name="fraw", bufs=2) as frawp,
        ):
            rec_pools = tc.tile_pool(name="ppre", bufs=2, space="PSUM")
            ppre = rec_pools.__enter__()
            pgate_mgr = tc.tile_pool(name="pgate", bufs=2, space="PSUM")
            pgate = pgate_mgr.__enter__()
            # ---- weight streams ---------------------------------------
            # All w2 chunks first (both queues, ~1us cadence, consumed by
            # ctxl at ~1 group/rec-step so the stream never backs up),
            # then all w1 slices (needed only by the FC tail).
            w2_tiles = [None] * NG

            def emit_w2load(g):
                vs = g * VG * 128
                eng = nc.sync if g % 2 == 0 else nc.gpsimd
                w2t = w2p.tile([128, KH, VG * 128], bf, tag="w2t")
                eng.dma_start(
                    out=w2t[:],
                    in_=w2_d[:, :, vs:vs + VG * 128].rearrange("k p n -> p k n"),
                )
                w2_tiles[g] = w2t

            def emit_w1load(g):
                vs = g * VG * 128
                eng = nc.sync if g % 2 == 0 else nc.gpsimd
                eng.dma_start(
                    out=w1_sb[:, :, vs:vs + VG * 128],
                    in_=w1_d[:, :, vs:vs + VG * 128].rearrange("k p n -> p k n"),
                )

            # ---- phase-1 pre block: 4 gate-quads x 6 k matmuls -------
            def emit_pre_block(blk):
                ts = blk * 128
                for q in range(4):
                    pp = ppre.tile([128, 4, 128], f32, tag="ppre")
                    for gt in range(4 * q, 4 * q + 4):
                        for kt in range(KI):
                            nc.tensor.matmul(
                                pp[:, gt - 4 * q],
                                lhsT=wih_sb[:, kt, gt * 128:(gt + 1) * 128],
                                rhs=x_sb[:, kt, ts:ts + 128],
                                start=(kt == 0),
                                stop=(kt == KI - 1),
                            )
                    nc.vector.tensor_add(
                        pre_sb[:, 4 * q:4 * q + 4, ts:ts + 128],
                        pp[:],
                        bias_sb[:, 4 * q:4 * q + 4]
                        .unsqueeze(2).broadcast_to([128, 4, 128]),
                    )

            # ---- ctxl group: ctxl[v, b] for VG vtiles ----------------
            def emit_ctxl_group(g):
                w2t = w2_tiles[g]
                pc = ppre.tile([128, VG, BL], f32, name="pc", tag="ppre")
                for v in range(VG):
                    for kt in range(KH):
                        nc.tensor.matmul(
                            pc[:, v],
                            lhsT=w2t[:, kt, v * 128:(v + 1) * 128],
                            rhs=ctxT_sb[:, kt, :],
                            start=(kt == 0),
                            stop=(kt == KH - 1),
                        )
                vs = g * VG
                nc.vector.tensor_add(
                    ctxl_sb[:, vs:vs + VG, :],
                    pc[:],
                    bfc_sb[:, vs:vs + VG].unsqueeze(2)
                    .broadcast_to([128, VG, BL]),
                )

            # ---- FC: one unit = a QUAD of vtiles over all 512 local
            # tokens.  8 DR matmuls into a 4-bank psum tile, then either
            # (lane A) one DVE tensor_add evac with the broadcast ctx
            # term, or (lane B) an Act copy to SBUF + a Pool SBUF-only
            # add (GPSIMD cannot touch PSUM).  One 4KB store per quad.
            def emit_fc_quad(q, lane, st):
                v0 = 4 * q
                ps = pfc.tile([128, 4, TOKL], f32, tag="pfc")
                for j in range(4):
                    for kq in range(2):
                        nc.tensor.matmul(
                            ps[:, j],
                            lhsT=w1_sb[:, 2 * kq:2 * kq + 2,
                                       (v0 + j) * 128:(v0 + j + 1) * 128],
                            rhs=hs_sb[:, 2 * kq:2 * kq + 2, :],
                            perf_mode=DR,
                            start=(kq == 0),
                            stop=(kq == 1),
                        )
                fo = foutp.tile([128, 4, TOKL], bf, name="fo", tag="fo")
                ctxv = (ctxl_sb[:, v0:v0 + 4, :].unsqueeze(2)
                        .broadcast_to([128, 4, TOKL // BL, BL]))
                psv = ps[:].rearrange("p v (t b) -> p v t b", b=BL)
                fov = fo[:].rearrange("p v (t b) -> p v t b", b=BL)
                if lane == 0:
                    nc.vector.tensor_add(fov, psv, ctxv)
                else:
                    fr = frawp.tile([128, 4, TOKL], bf, name="fr", tag="fr")
                    nc.scalar.copy(fr[:], ps[:])
                    nc.gpsimd.tensor_add(
                        fov, fr[:].rearrange("p v (t b) -> p v t b", b=BL),
                        ctxv)
                st.dma_start(
                    out=log_d[v0:v0 + 4, :, :].rearrange("v p n -> p v n"),
                    in_=fo[:],
                )

            # ---- filler schedule: extra work emitted per rec step ----
            # step -> list of thunks
            filler = [[] for _ in range(S + 1)]
            for blk in range(1, 4):
                filler[2 * (blk - 1) + 2].append(
                    lambda blk=blk: emit_pre_block(blk))
            # ctxl group g consumes w2 chunk g: chunk cadence ~1us on
            # two dedicated queues < rec step ~1.05us, so pacing one
            # group per step (6 steps of slack) never stalls the PE queue
            for g in range(NG):
                filler[min(6 + g, S - 1)].append(
                    lambda g=g: emit_ctxl_group(g))
            # FC half-0 pairs, 3 per step from step 40 (Pool/SP are
            # draining the tail of the weight stream by then; stores go
            # to SP only, which has no recurrence-critical work left)

            # weight loads are pure DMA: emit them all up front so the
            # queues start immediately.  w1-g rides right behind w2-g so
            # the FC fillers (which need early w1 groups) are never
            # starved by the consumption-gated w2 stream.
            for g in range(NG):
                emit_w2load(g)
            for g in range(NG):
                emit_w1load(g)

            # ---- phase-1 block 0 then the recurrence -----------------
            emit_pre_block(0)

            c_prev = c0_sb
            for t in range(S):
                gp = pgate.tile([128, GT, BL], f32, tag="gates")
                # pre contribution via identity matmul (accumulate base)
                nc.tensor.matmul(
                    gp[:],
                    lhsT=id_sb[:],
                    rhs=pre_sb[:, :, t * BL:(t + 1) * BL],
                    start=True,
                    stop=False,
                )
                rhs_src = h0_sb if t == 0 else hs_sb
                roff = 0 if t == 0 else (t - 1) * BL
                for gt in range(GT):
                    for kq in range(2):
                        nc.tensor.matmul(
                            gp[:, gt],
                            lhsT=whh_sb[:, 2 * kq:2 * kq + 2,
                                        gt * 128:(gt + 1) * 128],
                            rhs=rhs_src[:, 2 * kq:2 * kq + 2,
                                        roff:roff + BL],
                            perf_mode=DR,
                            start=False,
                            stop=(gt == GT - 1 and kq == 1),
                        )
                # activations: [i, f, o | g] after host permute
                sif = actp.tile([128, 3 * KH, BL], bf, tag="sif")
                gg = actp.tile([128, KH, BL], bf, tag="gg")
                nc.scalar.activation(sif[:], gp[:, 0:3 * KH], AF.Sigmoid)
                nc.scalar.activation(gg[:], gp[:, 3 * KH:GT], AF.Tanh)

                t1 = tmpp.tile([128, KH, BL], f32, tag="t1")
                t2 = tmpp.tile([128, KH, BL], f32, tag="t2")
                c_new = cstp.tile([128, KH, BL], f32, tag="c")
                tcn = actp.tile([128, KH, BL], bf, tag="tc")
                nc.vector.tensor_mul(t1[:], sif[:, 0:KH], gg[:])
                nc.vector.tensor_mul(t2[:], sif[:, KH:2 * KH], c_prev[:])
                nc.vector.tensor_add(c_new[:], t1[:], t2[:])
                nc.scalar.activation(tcn[:], c_new[:], AF.Tanh)
                # h (fp8) in two k-pair halves so step t+1's first DR
                # matmul can start before the second half lands
                hslice = hs_sb[:, :, t * BL:(t + 1) * BL]
                nc.vector.tensor_mul(
                    hslice[:, 0:2], sif[:, 2 * KH:2 * KH + 2], tcn[:, 0:2])
                nc.vector.tensor_mul(
                    hslice[:, 2:4], sif[:, 2 * KH + 2:3 * KH], tcn[:, 2:4])
                c_prev = c_new

                for th in filler[t]:
                    th()

            # ---- tail: the vocab projection ---------------------------
            pgate_mgr.__exit__(None, None, None)
            rec_pools.__exit__(None, None, None)
            with tc.tile_pool(name="pfc", bufs=2, space="PSUM") as pfc:
                lanes = [0, 1]
                stores = [nc.sync, nc.sync, nc.sync, nc.sync, nc.gpsimd,
                          nc.scalar]
                for q in range(VT // 4):
                    emit_fc_quad(q, lanes[q % 2], stores[q % 6])
                # last 2 vtiles (250 = 4*62 + 2)
                v0 = VT - 2
                ps = pfc.tile([128, 2, TOKL], f32, name="ps2", tag="pfc")
                for j in range(2):
                    for kq in range(2):
                        nc.tensor.matmul(
                            ps[:, j],
                            lhsT=w1_sb[:, 2 * kq:2 * kq + 2,
                                       (v0 + j) * 128:(v0 + j + 1) * 128],
                            rhs=hs_sb[:, 2 * kq:2 * kq + 2, :],
                            perf_mode=DR,
                            start=(kq == 0),
                            stop=(kq == 1),
                        )
                fo = foutp.tile([128, 2, TOKL], bf, name="fo2", tag="fo")
                nc.vector.tensor_add(
                    fo[:].rearrange("p v (t b) -> p v t b", b=BL),
                    ps[:].rearrange("p v (t b) -> p v t b", b=BL),
                    ctxl_sb[:, v0:v0 + 2, :].unsqueeze(2)
                    .broadcast_to([128, 2, TOKL // BL, BL]),
                )
                nc.sync.dma_start(
                    out=log_d[v0:v0 + 2, :, :].rearrange("v p n -> p v n"),
                    in_=fo[:],
                )

    nc.compile()
    return nc


def _get_nc():
    if "nc" not in _CACHE:
        _CACHE["nc"] = _build_program()
    return _CACHE["nc"]


def _block128(a):
    """[K, N] -> [K//128, 128, N] contiguous blocks."""
    k, n = a.shape
    return np.ascontiguousarray(a.reshape(k // 128, 128, n))


def _t_layout(a, dt):
    """[BL, 512] state -> [128, KH, BL] transposed tile layout."""
    return np.ascontiguousarray(a.T.reshape(KH, 128, BL).transpose(1, 0, 2)
                                ).astype(dt)


def _prep_in_maps(target_seq, context, h, c, emb_table, w_ih, w_hh, b_ih,
                  b_hh, w_fc, b_fc):
    target_seq = np.asarray(target_seq)
    context = np.asarray(context, dtype=np.float32)
    h = np.asarray(h, dtype=np.float32)
    c = np.asarray(c, dtype=np.float32)
    emb_table = np.asarray(emb_table, dtype=np.float32)
    w_ih = np.asarray(w_ih, dtype=np.float32)
    w_hh = np.asarray(w_hh, dtype=np.float32)
    b_ih = np.asarray(b_ih, dtype=np.float32)
    b_hh = np.asarray(b_hh, dtype=np.float32)
    w_fc = np.asarray(w_fc, dtype=np.float32)
    b_fc = np.asarray(b_fc, dtype=np.float32)

    # gate-block permutation [i, f, g, o] -> [i, f, o, g]
    perm = np.concatenate([
        np.arange(0, HIDDEN),                    # i
        np.arange(HIDDEN, 2 * HIDDEN),           # f
        np.arange(3 * HIDDEN, 4 * HIDDEN),       # o
        np.arange(2 * HIDDEN, 3 * HIDDEN),       # g
    ])
    w_ih_p = w_ih[perm]
    w_hh_p = w_hh[perm]
    bias_p = (b_ih + b_hh)[perm]

    wih_d = _block128(w_ih_p.T.astype(FP8))            # [6,128,2048]
    whh_d = _block128(w_hh_p.T.astype(FP8))            # [4,128,2048]
    bias_d = np.ascontiguousarray(
        bias_p.reshape(GT, 128).T.astype(np.float32))  # [128,16]
    id_d = np.eye(128, dtype=FP8)
    w1_d = _block128(np.ascontiguousarray(w_fc[:, :HIDDEN].T).astype(FP8))
    w2_d = _block128(np.ascontiguousarray(w_fc[:, HIDDEN:].T).astype(BF16))
    bfc_d = np.ascontiguousarray(
        b_fc.reshape(VT, 128).T.astype(np.float32))    # [128,250]

    emb = emb_table[target_seq]                        # [B,S,E] f32

    in_maps = []
    for cid in range(NCORES):
        bs = slice(cid * BL, (cid + 1) * BL)
        # lstm_in transposed, local tokens n = t*8+b
        x_loc = np.concatenate(
            [
                emb[bs].transpose(1, 0, 2).reshape(TOKL, EMBED),
                np.tile(context[bs], (S, 1)),
            ],
            axis=1,
        )                                              # [512, 768]
        x_d = _block128(x_loc.T.astype(BF16))          # [6,128,512]
        ctxT_d = _block128(context[bs].T.astype(BF16))  # [4,128,8]
        in_maps.append({
            "x_d": x_d,
            "wih_d": wih_d,
            "whh_d": whh_d,
            "bias_d": bias_d,
            "h0_d": _t_layout(h[bs], FP8),
            "c0_d": _t_layout(c[bs], np.float32),
            "ctxT_d": ctxT_d,
            "id_d": id_d,
            "w1_d": w1_d,
            "w2_d": w2_d,
            "bfc_d": bfc_d,
        })
    return in_maps


def _assemble(results):
    """results: list of per-core {"log_d": [250, 128, 512]} -> [B, S, V]."""
    logits = np.empty((B, S, VOCAB), dtype=np.float32)
    for cid in range(NCORES):
        out_c = results[cid]["log_d"].reshape(VOCAB, S, BL)  # [V, t, b]
        logits[cid * BL:(cid + 1) * BL] = (
            out_c.transpose(2, 1, 0).astype(np.float32))
    return np.ascontiguousarray(logits)


def kernel(target_seq, context, h, c, emb_table, w_ih, w_hh, b_ih, b_hh,
           w_fc, b_fc):
    from concourse.bass_utils import run_bass_kernel_spmd

    in_maps = _prep_in_maps(target_seq, context, h, c, emb_table, w_ih,
                            w_hh, b_ih, b_hh, w_fc, b_fc)
    nc = _get_nc()
    res = run_bass_kernel_spmd(nc, in_maps, core_ids=list(range(NCORES)))
    return _assemble(res.results)
